# revision 15
# baseline (speedup 1.0000x reference)
"""Trainium2 Bass kernel for AudioAdapterAttnProcessor.

Reference computation (B=4, S=4096, D=1024, H=16, HD=64, C=768,
S_TXT=77, S_AUD=16):
    q = (hidden @ Wq)                                  [B, S, H, HD]
    base  = softmax(q k_t^T / 8) v_t   (text cross-attn, k/v from encoder)
    audio = softmax(q k_a^T / 8) v_a   (audio cross-attn)
    out = concat_heads(base + audio) @ Wo + bo

Sharding: (batch x seq-half) -> 8 cores; each core handles one batch's
2048 queries for all 16 heads.  No collectives: every query row of the
output depends only on its own hidden row (cross-attention to 93 fixed
keys per batch), so the gather is a pure concatenation.

Per-core device program (everything transposed so PE contracts naturally):
    qT   = Wq^T @ hiddenT                (hiddenT fed pre-transposed, bf16)
    s    = q_h^T.T @ kcatT_h             natural scores [128q, 93keys]
    p    = exp(s/8) (fused scale); denominators via ACT accum_out
    pn   = p * recip(den)                per-partition tensor_scalar
    pT   = PE-transpose(pn)              [93, 128] bf16
    pv   = v_h(pad128)^T.T @ pT          -> attn_outT [64, 512] per head
    out  = attn_T.T @ Wo                 natural [128q, 1024] fp32 -> DRAM
bo is added on the host during the gather (it is per-output-feature,
which is the free dim of the natural-layout output).
"""

import sys

sys.path.insert(0, "/opt/trn_rl_repo")

from contextlib import ExitStack

import numpy as np
import ml_dtypes

import concourse.bass as bass
import concourse.mybir as mybir
import concourse.tile as tile
from concourse import bacc
from concourse.masks import make_identity

BF16 = ml_dtypes.bfloat16

B, S, D = 4, 4096, 1024
S_TXT, S_AUD = 77, 16
C = 768
H = 16
HD = 64
NK = S_TXT + S_AUD  # 93 keys after concat
P = 128
SCALE = 1.0 / np.sqrt(HD)  # 0.125

N_CORES = 8
SEQ_PER_CORE = S // 2  # 2048
CHUNK = 512
N_CHUNKS = SEQ_PER_CORE // CHUNK  # 4
KT = D // P  # 8 contraction tiles
QT_PER_CHUNK = CHUNK // P  # 4


DEFAULT_CFG = dict(hidden=2, qT=2, probs_u=3, probs_n=3, dsum=2, attn_T=2,
                   out_sb=3, accps=3, sps=2, tps=2, vps=1)


def build_bass(cfg=None):
    """Build the SPMD single-core Bass program (same program on all 8 cores)."""
    cfg = {**DEFAULT_CFG, **(cfg or {})}
    nc = bacc.Bacc("TRN2", target_bir_lowering=False, debug=False, num_devices=N_CORES)

    ht_d = nc.dram_tensor("ht", [P, KT * SEQ_PER_CORE], mybir.dt.bfloat16, kind="ExternalInput")
    wq_d = nc.dram_tensor("wq", [P, KT * D], mybir.dt.bfloat16, kind="ExternalInput")
    wo_d = nc.dram_tensor("wo", [P, KT * D], mybir.dt.bfloat16, kind="ExternalInput")
    kc_d = nc.dram_tensor("kc", [P, KT * NK], mybir.dt.bfloat16, kind="ExternalInput")
    v_d = nc.dram_tensor("v", [P, D], mybir.dt.bfloat16, kind="ExternalInput")
    out_d = nc.dram_tensor("out", [SEQ_PER_CORE, D], mybir.dt.float32, kind="ExternalOutput")

    ht_view = ht_d[:].rearrange("p (k q) -> p k q", k=KT)  # [128, 8, 2048]

    with tile.TileContext(nc) as tc, ExitStack() as ctx:
        wpool = ctx.enter_context(tc.tile_pool(name="weights", bufs=1))
        hpool = ctx.enter_context(tc.tile_pool(name="hidden", bufs=cfg["hidden"]))
        qpool = ctx.enter_context(tc.tile_pool(name="qT", bufs=cfg["qT"]))
        ppool = ctx.enter_context(tc.tile_pool(name="probs_u", bufs=cfg["probs_u"]))
        pnpool = ctx.enter_context(tc.tile_pool(name="probs_n", bufs=cfg["probs_n"]))
        dpool = ctx.enter_context(tc.tile_pool(name="dsum", bufs=cfg["dsum"]))
        atpool = ctx.enter_context(tc.tile_pool(name="attn_T", bufs=cfg["attn_T"]))
        opool = ctx.enter_context(tc.tile_pool(name="out_sb", bufs=cfg["out_sb"]))

        accps = ctx.enter_context(tc.tile_pool(name="accps", bufs=cfg["accps"], space="PSUM"))
        sps = ctx.enter_context(tc.tile_pool(name="sps", bufs=cfg["sps"], space="PSUM"))
        tps = ctx.enter_context(tc.tile_pool(name="tps", bufs=cfg["tps"], space="PSUM"))
        vps = ctx.enter_context(tc.tile_pool(name="vps", bufs=cfg["vps"], space="PSUM"))

        # One-time loads
        wq_t = wpool.tile([P, KT * D], mybir.dt.bfloat16)
        wo_t = wpool.tile([P, KT * D], mybir.dt.bfloat16)
        kc_t = wpool.tile([P, KT * NK], mybir.dt.bfloat16)
        v_t = wpool.tile([P, D], mybir.dt.bfloat16)
        ident = wpool.tile([P, P], mybir.dt.bfloat16)
        nc.sync.dma_start(wq_t[:], wq_d[:])
        nc.sync.dma_start(wo_t[:], wo_d[:])
        nc.sync.dma_start(kc_t[:], kc_d[:])
        nc.sync.dma_start(v_t[:], v_d[:])
        make_identity(nc, ident[:])

        # Persistent transposed-probs buffer; pad rows zeroed once so the
        # K=128 PV matmul never multiplies v's zero pad rows by NaN garbage.
        expT = wpool.tile([P, H, CHUNK], mybir.dt.bfloat16)
        nc.gpsimd.memset(expT[:], 0.0)

        for c in range(N_CHUNKS):
            ht_t = hpool.tile([P, KT, CHUNK], mybir.dt.bfloat16)
            nc.sync.dma_start(ht_t[:], ht_view[:, :, c * CHUNK : (c + 1) * CHUNK])

            # ---- q projection: qT[:, mt, :] = Wq[:, mt-block]^T @ hiddenT ----
            qT_t = qpool.tile([P, KT, CHUNK], mybir.dt.bfloat16)
            for mt in range(KT):
                qp = accps.tile([P, CHUNK], mybir.dt.float32, tag="acc")
                for kt in range(KT):
                    nc.tensor.matmul(
                        qp[:],
                        lhsT=wq_t[:, kt * D + mt * P : kt * D + (mt + 1) * P],
                        rhs=ht_t[:, kt, :],
                        start=(kt == 0),
                        stop=(kt == KT - 1),
                    )
                nc.scalar.copy(qT_t[:, mt, :], qp[:])

            # ---- attention per head ----
            # 8 denominators per head: 4 q-tiles x (text, audio)
            dsum = dpool.tile([P, H * 8], mybir.dt.float32)
            rds = dpool.tile([P, H * 8], mybir.dt.float32, tag="rds")
            at_t = atpool.tile([P, KT, CHUNK], mybir.dt.bfloat16, tag="at")
            for g in range(H // 2):  # head pairs (2g, 2g+1)
                tpp = tps.tile([NK, 2 * CHUNK], mybir.dt.bfloat16)
                pvp = vps.tile([P, CHUNK], mybir.dt.float32)
                for hh in range(2):
                    h = 2 * g + hh
                    off = hh * HD
                    d0 = h * 8
                    sp = sps.tile([P, QT_PER_CHUNK * NK], mybir.dt.float32)
                    pu = ppool.tile([P, QT_PER_CHUNK * NK], mybir.dt.float32)
                    pn = pnpool.tile([P, QT_PER_CHUNK * NK], mybir.dt.bfloat16)
                    for qt in range(QT_PER_CHUNK):
                        nc.tensor.matmul(
                            sp[:, qt * NK : (qt + 1) * NK],
                            lhsT=qT_t[off : off + HD, g, qt * P : (qt + 1) * P],
                            rhs=kc_t[off : off + HD, g * NK : (g + 1) * NK],
                            start=True,
                            stop=True,
                        )
                    # one fused exp per head; denominators via segmented reduces
                    nc.scalar.activation(
                        pu[:],
                        sp[:],
                        mybir.ActivationFunctionType.Exp,
                        scale=float(SCALE),
                    )
                    pu3 = pu[:].rearrange("p (q k) -> p q k", q=QT_PER_CHUNK)
                    nc.vector.reduce_sum(
                        dsum[:, d0 : d0 + 4], pu3[:, :, 0:S_TXT],
                        axis=mybir.AxisListType.X,
                    )
                    nc.vector.reduce_sum(
                        dsum[:, d0 + 4 : d0 + 8], pu3[:, :, S_TXT:NK],
                        axis=mybir.AxisListType.X,
                    )
                    nc.vector.reciprocal(rds[:, d0 : d0 + 8], dsum[:, d0 : d0 + 8])
                    for qt in range(QT_PER_CHUNK):
                        nc.vector.tensor_scalar_mul(
                            pn[:, qt * NK : qt * NK + S_TXT],
                            pu[:, qt * NK : qt * NK + S_TXT],
                            rds[:, d0 + qt : d0 + qt + 1],
                        )
                        nc.vector.tensor_scalar_mul(
                            pn[:, qt * NK + S_TXT : (qt + 1) * NK],
                            pu[:, qt * NK + S_TXT : (qt + 1) * NK],
                            rds[:, d0 + 4 + qt : d0 + 4 + qt + 1],
                        )
                    for qt in range(QT_PER_CHUNK):
                        nc.tensor.transpose(
                            tpp[:, hh * CHUNK + qt * P : hh * CHUNK + (qt + 1) * P],
                            pn[:, qt * NK : (qt + 1) * NK],
                            ident[:],
                        )
                # one copy moves both heads' transposed probs to SBUF
                nc.scalar.copy(expT[0:NK, 2 * g : 2 * g + 2, :], tpp[:])
                for hh in range(2):
                    h = 2 * g + hh
                    off = hh * HD
                    nc.tensor.matmul(
                        pvp[off : off + HD, :],
                        lhsT=v_t[:, h * HD : (h + 1) * HD],
                        rhs=expT[:, h, :],
                        start=True,
                        stop=True,
                    )
                # attn_T tile g holds heads 2g (rows 0-63) and 2g+1 (64-127)
                nc.scalar.copy(at_t[:, g, :], pvp[:])

            # ---- output projection: out[q, :] = attn_T.T @ Wo ----
            for qt in range(QT_PER_CHUNK):
                for nb in range(2):
                    op = accps.tile([P, CHUNK], mybir.dt.float32, tag="acc")
                    for kt in range(KT):
                        nc.tensor.matmul(
                            op[:],
                            lhsT=at_t[:, kt, qt * P : (qt + 1) * P],
                            rhs=wo_t[:, kt * D + nb * CHUNK : kt * D + (nb + 1) * CHUNK],
                            start=(kt == 0),
                            stop=(kt == KT - 1),
                        )
                    ob = opool.tile([P, CHUNK], mybir.dt.float32)
                    nc.vector.tensor_copy(ob[:], op[:])
                    nc.sync.dma_start(
                        out_d[
                            c * CHUNK + qt * P : c * CHUNK + (qt + 1) * P,
                            nb * CHUNK : (nb + 1) * CHUNK,
                        ],
                        ob[:],
                    )

    nc.compile()
    return nc


def _host_prep(hidden_states, encoder_hidden_states, audio_hidden_states,
               Wq, Wk, Wv, Wk_audio, Wv_audio, Wo):
    """Build the per-core input maps (all layouts pre-arranged on host)."""
    wq_sb = np.ascontiguousarray(
        Wq.reshape(KT, P, D).transpose(1, 0, 2).reshape(P, KT * D)
    ).astype(BF16)
    wo_sb = np.ascontiguousarray(
        Wo.reshape(KT, P, D).transpose(1, 0, 2).reshape(P, KT * D)
    ).astype(BF16)

    in_maps = []
    for b in range(B):
        # kv projections for this batch: tiny, done on host
        k_full = np.concatenate(
            [encoder_hidden_states[b] @ Wk, audio_hidden_states[b] @ Wk_audio], axis=0
        )  # [93, 1024]
        v_full = np.concatenate(
            [encoder_hidden_states[b] @ Wv, audio_hidden_states[b] @ Wv_audio], axis=0
        )  # [93, 1024]
        kc_sb = np.ascontiguousarray(
            k_full.T.reshape(KT, P, NK).transpose(1, 0, 2).reshape(P, KT * NK)
        ).astype(BF16)
        v_sb = np.zeros((P, D), dtype=BF16)
        v_sb[:NK] = v_full.astype(BF16)

        for half in range(2):
            rows = hidden_states[b, half * SEQ_PER_CORE : (half + 1) * SEQ_PER_CORE]
            ht_sb = np.ascontiguousarray(
                rows.T.reshape(KT, P, SEQ_PER_CORE)
                .transpose(1, 0, 2)
                .reshape(P, KT * SEQ_PER_CORE)
            ).astype(BF16)
            in_maps.append(
                {"ht": ht_sb, "wq": wq_sb, "wo": wo_sb, "kc": kc_sb, "v": v_sb}
            )
    return in_maps


_NC_CACHE = {}


def get_nc():
    if "nc" not in _NC_CACHE:
        _NC_CACHE["nc"] = build_bass()
    return _NC_CACHE["nc"]


def kernel(hidden_states, encoder_hidden_states, audio_hidden_states,
           Wq, Wk, Wv, Wk_audio, Wv_audio, Wo, bo):
    from concourse import bass_utils

    hidden_states = np.asarray(hidden_states, dtype=np.float32)
    encoder_hidden_states = np.asarray(encoder_hidden_states, dtype=np.float32)
    audio_hidden_states = np.asarray(audio_hidden_states, dtype=np.float32)
    Wq = np.asarray(Wq, dtype=np.float32)
    Wk = np.asarray(Wk, dtype=np.float32)
    Wv = np.asarray(Wv, dtype=np.float32)
    Wk_audio = np.asarray(Wk_audio, dtype=np.float32)
    Wv_audio = np.asarray(Wv_audio, dtype=np.float32)
    Wo = np.asarray(Wo, dtype=np.float32)
    bo = np.asarray(bo, dtype=np.float32)

    nc = get_nc()
    in_maps = _host_prep(hidden_states, encoder_hidden_states, audio_hidden_states,
                         Wq, Wk, Wv, Wk_audio, Wv_audio, Wo)
    res = bass_utils.run_bass_kernel_spmd(nc, in_maps, list(range(N_CORES)))

    out = np.empty((B, S, D), dtype=np.float32)
    core = 0
    for b in range(B):
        for half in range(2):
            out[b, half * SEQ_PER_CORE : (half + 1) * SEQ_PER_CORE] = res.results[core]["out"]
            core += 1
    out += bo[None, None, :]
    return out


# revision 17
# speedup vs baseline: 129.7498x; 129.7498x over previous
"""Trainium2 Bass kernel for AudioAdapterAttnProcessor.

Reference computation (B=4, S=4096, D=1024, H=16, HD=64, C=768,
S_TXT=77, S_AUD=16):
    q = (hidden @ Wq)                                  [B, S, H, HD]
    base  = softmax(q k_t^T / 8) v_t   (text cross-attn, k/v from encoder)
    audio = softmax(q k_a^T / 8) v_a   (audio cross-attn)
    out = concat_heads(base + audio) @ Wo + bo

Sharding: (batch x seq-half) -> 8 cores; each core handles one batch's
2048 queries for all 16 heads.  No collectives: every query row of the
output depends only on its own hidden row (cross-attention to 93 fixed
keys per batch), so the gather is a pure concatenation.

Per-core device program (everything transposed so PE contracts naturally):
    qT   = Wq^T @ hiddenT                (hiddenT fed pre-transposed, bf16)
    s    = q_h^T.T @ kcatT_h             natural scores [128q, 93keys]
    p    = exp(s/8) (fused scale); denominators via ACT accum_out
    pn   = p * recip(den)                per-partition tensor_scalar
    pT   = PE-transpose(pn)              [93, 128] bf16
    pv   = v_h(pad128)^T.T @ pT          -> attn_outT [64, 512] per head
    out  = attn_T.T @ Wo                 natural [128q, 1024] fp32 -> DRAM
bo is added on the host during the gather (it is per-output-feature,
which is the free dim of the natural-layout output).
"""

import sys

sys.path.insert(0, "/opt/trn_rl_repo")

from contextlib import ExitStack

import numpy as np
import ml_dtypes

import concourse.bass as bass
import concourse.mybir as mybir
import concourse.tile as tile
from concourse import bacc
from concourse.masks import make_identity

BF16 = ml_dtypes.bfloat16

B, S, D = 4, 4096, 1024
S_TXT, S_AUD = 77, 16
C = 768
H = 16
HD = 64
NK = S_TXT + S_AUD  # 93 keys after concat
P = 128
SCALE = 1.0 / np.sqrt(HD)  # 0.125

N_CORES = 8
SEQ_PER_CORE = S // 2  # 2048
CHUNK = 512
N_CHUNKS = SEQ_PER_CORE // CHUNK  # 4
KT = D // P  # 8 contraction tiles
QT_PER_CHUNK = CHUNK // P  # 4


DEFAULT_CFG = dict(hidden=2, qT=2, probs_u=3, probs_n=3, dsum=2, attn_T=2,
                   out_sb=3, accps=3, sps=2, tps=2, vps=1)


def build_bass(cfg=None, reps=1):
    """Build the SPMD single-core Bass program (same program on all 8 cores).

    reps > 1 repeats the whole computation back-to-back inside the NEFF
    (same inputs -> same outputs); used only for slope-based timing.
    """
    cfg = {**DEFAULT_CFG, **(cfg or {})}
    nc = bacc.Bacc("TRN2", target_bir_lowering=False, debug=False, num_devices=N_CORES)

    ht_d = nc.dram_tensor("ht", [P, KT * SEQ_PER_CORE], mybir.dt.bfloat16, kind="ExternalInput")
    wq_d = nc.dram_tensor("wq", [P, KT * D], mybir.dt.bfloat16, kind="ExternalInput")
    wo_d = nc.dram_tensor("wo", [P, KT * D], mybir.dt.bfloat16, kind="ExternalInput")
    kc_d = nc.dram_tensor("kc", [P, KT * NK], mybir.dt.bfloat16, kind="ExternalInput")
    v_d = nc.dram_tensor("v", [P, D], mybir.dt.bfloat16, kind="ExternalInput")
    out_d = nc.dram_tensor("out", [SEQ_PER_CORE, D], mybir.dt.float32, kind="ExternalOutput")

    ht_view = ht_d[:].rearrange("p (k q) -> p k q", k=KT)  # [128, 8, 2048]

    with tile.TileContext(nc) as tc, ExitStack() as ctx:
        wpool = ctx.enter_context(tc.tile_pool(name="weights", bufs=1))
        hpool = ctx.enter_context(tc.tile_pool(name="hidden", bufs=cfg["hidden"]))
        qpool = ctx.enter_context(tc.tile_pool(name="qT", bufs=cfg["qT"]))
        ppool = ctx.enter_context(tc.tile_pool(name="probs_u", bufs=cfg["probs_u"]))
        pnpool = ctx.enter_context(tc.tile_pool(name="probs_n", bufs=cfg["probs_n"]))
        dpool = ctx.enter_context(tc.tile_pool(name="dsum", bufs=cfg["dsum"]))
        atpool = ctx.enter_context(tc.tile_pool(name="attn_T", bufs=cfg["attn_T"]))
        opool = ctx.enter_context(tc.tile_pool(name="out_sb", bufs=cfg["out_sb"]))

        accps = ctx.enter_context(tc.tile_pool(name="accps", bufs=cfg["accps"], space="PSUM"))
        sps = ctx.enter_context(tc.tile_pool(name="sps", bufs=cfg["sps"], space="PSUM"))
        tps = ctx.enter_context(tc.tile_pool(name="tps", bufs=cfg["tps"], space="PSUM"))
        vps = ctx.enter_context(tc.tile_pool(name="vps", bufs=cfg["vps"], space="PSUM"))

        # One-time loads
        wq_t = wpool.tile([P, KT * D], mybir.dt.bfloat16)
        wo_t = wpool.tile([P, KT * D], mybir.dt.bfloat16)
        kc_t = wpool.tile([P, KT * NK], mybir.dt.bfloat16)
        v_t = wpool.tile([P, D], mybir.dt.bfloat16)
        ident = wpool.tile([P, P], mybir.dt.bfloat16)
        nc.sync.dma_start(wq_t[:], wq_d[:])
        nc.sync.dma_start(wo_t[:], wo_d[:])
        nc.sync.dma_start(kc_t[:], kc_d[:])
        nc.sync.dma_start(v_t[:], v_d[:])
        make_identity(nc, ident[:])

        # Persistent transposed-probs buffer; pad rows zeroed once so the
        # K=128 PV matmul never multiplies v's zero pad rows by NaN garbage.
        expT = wpool.tile([P, H, CHUNK], mybir.dt.bfloat16)
        nc.gpsimd.memset(expT[:], 0.0)

        for c in [c for _ in range(reps) for c in range(N_CHUNKS)]:
            ht_t = hpool.tile([P, KT, CHUNK], mybir.dt.bfloat16)
            nc.sync.dma_start(ht_t[:], ht_view[:, :, c * CHUNK : (c + 1) * CHUNK])

            # ---- q projection: qT[:, mt, :] = Wq[:, mt-block]^T @ hiddenT ----
            qT_t = qpool.tile([P, KT, CHUNK], mybir.dt.bfloat16)
            for mt in range(KT):
                qp = accps.tile([P, CHUNK], mybir.dt.float32, tag="acc")
                for kt in range(KT):
                    nc.tensor.matmul(
                        qp[:],
                        lhsT=wq_t[:, kt * D + mt * P : kt * D + (mt + 1) * P],
                        rhs=ht_t[:, kt, :],
                        start=(kt == 0),
                        stop=(kt == KT - 1),
                    )
                nc.scalar.copy(qT_t[:, mt, :], qp[:])

            # ---- attention per head ----
            # 8 denominators per head: 4 q-tiles x (text, audio)
            dsum = dpool.tile([P, H * 8], mybir.dt.float32)
            rds = dpool.tile([P, H * 8], mybir.dt.float32, tag="rds")
            at_t = atpool.tile([P, KT, CHUNK], mybir.dt.bfloat16, tag="at")
            for g in range(H // 2):  # head pairs (2g, 2g+1)
                tpp = tps.tile([NK, 2 * CHUNK], mybir.dt.bfloat16)
                pvp = vps.tile([P, CHUNK], mybir.dt.float32)
                for hh in range(2):
                    h = 2 * g + hh
                    off = hh * HD
                    d0 = h * 8
                    sp = sps.tile([P, QT_PER_CHUNK * NK], mybir.dt.float32)
                    pu = ppool.tile([P, QT_PER_CHUNK * NK], mybir.dt.float32)
                    pn = pnpool.tile([P, QT_PER_CHUNK * NK], mybir.dt.bfloat16)
                    for qt in range(QT_PER_CHUNK):
                        nc.tensor.matmul(
                            sp[:, qt * NK : (qt + 1) * NK],
                            lhsT=qT_t[off : off + HD, g, qt * P : (qt + 1) * P],
                            rhs=kc_t[off : off + HD, g * NK : (g + 1) * NK],
                            start=True,
                            stop=True,
                        )
                    # one fused exp per head; denominators via segmented reduces
                    nc.scalar.activation(
                        pu[:],
                        sp[:],
                        mybir.ActivationFunctionType.Exp,
                        scale=float(SCALE),
                    )
                    pu3 = pu[:].rearrange("p (q k) -> p q k", q=QT_PER_CHUNK)
                    nc.vector.reduce_sum(
                        dsum[:, d0 : d0 + 4], pu3[:, :, 0:S_TXT],
                        axis=mybir.AxisListType.X,
                    )
                    nc.vector.reduce_sum(
                        dsum[:, d0 + 4 : d0 + 8], pu3[:, :, S_TXT:NK],
                        axis=mybir.AxisListType.X,
                    )
                    nc.vector.reciprocal(rds[:, d0 : d0 + 8], dsum[:, d0 : d0 + 8])
                    for qt in range(QT_PER_CHUNK):
                        nc.vector.tensor_scalar_mul(
                            pn[:, qt * NK : qt * NK + S_TXT],
                            pu[:, qt * NK : qt * NK + S_TXT],
                            rds[:, d0 + qt : d0 + qt + 1],
                        )
                        nc.vector.tensor_scalar_mul(
                            pn[:, qt * NK + S_TXT : (qt + 1) * NK],
                            pu[:, qt * NK + S_TXT : (qt + 1) * NK],
                            rds[:, d0 + 4 + qt : d0 + 4 + qt + 1],
                        )
                    for qt in range(QT_PER_CHUNK):
                        nc.tensor.transpose(
                            tpp[:, hh * CHUNK + qt * P : hh * CHUNK + (qt + 1) * P],
                            pn[:, qt * NK : (qt + 1) * NK],
                            ident[:],
                        )
                # one copy moves both heads' transposed probs to SBUF
                nc.scalar.copy(expT[0:NK, 2 * g : 2 * g + 2, :], tpp[:])
                for hh in range(2):
                    h = 2 * g + hh
                    off = hh * HD
                    nc.tensor.matmul(
                        pvp[off : off + HD, :],
                        lhsT=v_t[:, h * HD : (h + 1) * HD],
                        rhs=expT[:, h, :],
                        start=True,
                        stop=True,
                    )
                # attn_T tile g holds heads 2g (rows 0-63) and 2g+1 (64-127)
                nc.scalar.copy(at_t[:, g, :], pvp[:])

            # ---- output projection: out[q, :] = attn_T.T @ Wo ----
            for qt in range(QT_PER_CHUNK):
                for nb in range(2):
                    op = accps.tile([P, CHUNK], mybir.dt.float32, tag="acc")
                    for kt in range(KT):
                        nc.tensor.matmul(
                            op[:],
                            lhsT=at_t[:, kt, qt * P : (qt + 1) * P],
                            rhs=wo_t[:, kt * D + nb * CHUNK : kt * D + (nb + 1) * CHUNK],
                            start=(kt == 0),
                            stop=(kt == KT - 1),
                        )
                    ob = opool.tile([P, CHUNK], mybir.dt.float32)
                    nc.vector.tensor_copy(ob[:], op[:])
                    nc.sync.dma_start(
                        out_d[
                            c * CHUNK + qt * P : c * CHUNK + (qt + 1) * P,
                            nb * CHUNK : (nb + 1) * CHUNK,
                        ],
                        ob[:],
                    )

    nc.compile()
    return nc


def _host_prep(hidden_states, encoder_hidden_states, audio_hidden_states,
               Wq, Wk, Wv, Wk_audio, Wv_audio, Wo):
    """Build the per-core input maps (all layouts pre-arranged on host)."""
    wq_sb = np.ascontiguousarray(
        Wq.reshape(KT, P, D).transpose(1, 0, 2).reshape(P, KT * D)
    ).astype(BF16)
    wo_sb = np.ascontiguousarray(
        Wo.reshape(KT, P, D).transpose(1, 0, 2).reshape(P, KT * D)
    ).astype(BF16)

    in_maps = []
    for b in range(B):
        # kv projections for this batch: tiny, done on host
        k_full = np.concatenate(
            [encoder_hidden_states[b] @ Wk, audio_hidden_states[b] @ Wk_audio], axis=0
        )  # [93, 1024]
        v_full = np.concatenate(
            [encoder_hidden_states[b] @ Wv, audio_hidden_states[b] @ Wv_audio], axis=0
        )  # [93, 1024]
        kc_sb = np.ascontiguousarray(
            k_full.T.reshape(KT, P, NK).transpose(1, 0, 2).reshape(P, KT * NK)
        ).astype(BF16)
        v_sb = np.zeros((P, D), dtype=BF16)
        v_sb[:NK] = v_full.astype(BF16)

        for half in range(2):
            rows = hidden_states[b, half * SEQ_PER_CORE : (half + 1) * SEQ_PER_CORE]
            ht_sb = np.ascontiguousarray(
                rows.T.reshape(KT, P, SEQ_PER_CORE)
                .transpose(1, 0, 2)
                .reshape(P, KT * SEQ_PER_CORE)
            ).astype(BF16)
            in_maps.append(
                {"ht": ht_sb, "wq": wq_sb, "wo": wo_sb, "kc": kc_sb, "v": v_sb}
            )
    return in_maps


_NC_CACHE = {}


def get_nc():
    if "nc" not in _NC_CACHE:
        _NC_CACHE["nc"] = build_bass()
    return _NC_CACHE["nc"]


def kernel(hidden_states, encoder_hidden_states, audio_hidden_states,
           Wq, Wk, Wv, Wk_audio, Wv_audio, Wo, bo):
    from concourse import bass_utils

    hidden_states = np.asarray(hidden_states, dtype=np.float32)
    encoder_hidden_states = np.asarray(encoder_hidden_states, dtype=np.float32)
    audio_hidden_states = np.asarray(audio_hidden_states, dtype=np.float32)
    Wq = np.asarray(Wq, dtype=np.float32)
    Wk = np.asarray(Wk, dtype=np.float32)
    Wv = np.asarray(Wv, dtype=np.float32)
    Wk_audio = np.asarray(Wk_audio, dtype=np.float32)
    Wv_audio = np.asarray(Wv_audio, dtype=np.float32)
    Wo = np.asarray(Wo, dtype=np.float32)
    bo = np.asarray(bo, dtype=np.float32)

    nc = get_nc()
    in_maps = _host_prep(hidden_states, encoder_hidden_states, audio_hidden_states,
                         Wq, Wk, Wv, Wk_audio, Wv_audio, Wo)
    res = bass_utils.run_bass_kernel_spmd(nc, in_maps, list(range(N_CORES)))

    out = np.empty((B, S, D), dtype=np.float32)
    core = 0
    for b in range(B):
        for half in range(2):
            out[b, half * SEQ_PER_CORE : (half + 1) * SEQ_PER_CORE] = res.results[core]["out"]
            core += 1
    out += bo[None, None, :]
    return out


# revision 19
# speedup vs baseline: 209.0204x; 1.6109x over previous
"""Trainium2 Bass kernel for AudioAdapterAttnProcessor.

Reference computation (B=4, S=4096, D=1024, H=16, HD=64, C=768,
S_TXT=77, S_AUD=16):
    q = (hidden @ Wq)                                  [B, S, H, HD]
    base  = softmax(q k_t^T / 8) v_t   (text cross-attn, k/v from encoder)
    audio = softmax(q k_a^T / 8) v_a   (audio cross-attn)
    out = concat_heads(base + audio) @ Wo + bo

Sharding: (batch x seq-half) -> 8 cores; each core handles one batch's
2048 queries for all 16 heads.  No collectives: every query row of the
output depends only on its own hidden row (cross-attention to 93 fixed
keys per batch), so the gather is a pure concatenation.

Per-core device program (everything transposed so PE contracts naturally):
    qT   = Wq^T @ hiddenT                (hiddenT fed pre-transposed, bf16)
    s    = q_h^T.T @ kcatT_h             natural scores [128q, 93keys]
    p    = exp(s/8) (fused scale); denominators via ACT accum_out
    pn   = p * recip(den)                per-partition tensor_scalar
    pT   = PE-transpose(pn)              [93, 128] bf16
    pv   = v_h(pad128)^T.T @ pT          -> attn_outT [64, 512] per head
    out  = attn_T.T @ Wo                 natural [128q, 1024] fp32 -> DRAM
bo is added on the host during the gather (it is per-output-feature,
which is the free dim of the natural-layout output).
"""

import sys

sys.path.insert(0, "/opt/trn_rl_repo")

from contextlib import ExitStack

import numpy as np
import ml_dtypes

import concourse.bass as bass
import concourse.mybir as mybir
import concourse.tile as tile
from concourse import bacc
from concourse.masks import make_identity

BF16 = ml_dtypes.bfloat16

B, S, D = 4, 4096, 1024
S_TXT, S_AUD = 77, 16
C = 768
H = 16
HD = 64
NK = S_TXT + S_AUD  # 93 keys after concat
P = 128
SCALE = 1.0 / np.sqrt(HD)  # 0.125

N_CORES = 8
SEQ_PER_CORE = S // 2  # 2048
CHUNK = 512
N_CHUNKS = SEQ_PER_CORE // CHUNK  # 4
KT = D // P  # 8 contraction tiles
QT_PER_CHUNK = CHUNK // P  # 4


DEFAULT_CFG = dict(hidden=2, qT=2, probs_u=3, probs_n=3, dsum=2, attn_T=2,
                   out_sb=3, accps=3, sps=2, tps=2, vps=1)


def build_bass(cfg=None, reps=1):
    """Build the SPMD single-core Bass program (same program on all 8 cores).

    reps > 1 repeats the whole computation back-to-back inside the NEFF
    (same inputs -> same outputs); used only for slope-based timing.
    """
    cfg = {**DEFAULT_CFG, **(cfg or {})}
    nc = bacc.Bacc("TRN2", target_bir_lowering=False, debug=False, num_devices=N_CORES)

    ht_d = nc.dram_tensor("ht", [P, KT * SEQ_PER_CORE], mybir.dt.bfloat16, kind="ExternalInput")
    wq_d = nc.dram_tensor("wq", [P, KT * D], mybir.dt.bfloat16, kind="ExternalInput")
    wo_d = nc.dram_tensor("wo", [P, KT * D], mybir.dt.bfloat16, kind="ExternalInput")
    kc_d = nc.dram_tensor("kc", [P, KT * NK], mybir.dt.bfloat16, kind="ExternalInput")
    v_d = nc.dram_tensor("v", [P, D], mybir.dt.bfloat16, kind="ExternalInput")
    out_d = nc.dram_tensor("out", [SEQ_PER_CORE, D], mybir.dt.float32, kind="ExternalOutput")

    ht_view = ht_d[:].rearrange("p (k q) -> p k q", k=KT)  # [128, 8, 2048]

    with tile.TileContext(nc) as tc, ExitStack() as ctx:
        wpool = ctx.enter_context(tc.tile_pool(name="weights", bufs=1))
        hpool = ctx.enter_context(tc.tile_pool(name="hidden", bufs=cfg["hidden"]))
        qpool = ctx.enter_context(tc.tile_pool(name="qT", bufs=cfg["qT"]))
        ppool = ctx.enter_context(tc.tile_pool(name="probs_u", bufs=cfg["probs_u"]))
        pnpool = ctx.enter_context(tc.tile_pool(name="probs_n", bufs=cfg["probs_n"]))
        dpool = ctx.enter_context(tc.tile_pool(name="dsum", bufs=cfg["dsum"]))
        atpool = ctx.enter_context(tc.tile_pool(name="attn_T", bufs=cfg["attn_T"]))
        opool = ctx.enter_context(tc.tile_pool(name="out_sb", bufs=cfg["out_sb"]))

        accps = ctx.enter_context(tc.tile_pool(name="accps", bufs=cfg["accps"], space="PSUM"))
        sps = ctx.enter_context(tc.tile_pool(name="sps", bufs=cfg["sps"], space="PSUM"))
        tps = ctx.enter_context(tc.tile_pool(name="tps", bufs=cfg["tps"], space="PSUM"))
        vps = ctx.enter_context(tc.tile_pool(name="vps", bufs=cfg["vps"], space="PSUM"))

        # One-time loads
        wq_t = wpool.tile([P, KT * D], mybir.dt.bfloat16)
        wo_t = wpool.tile([P, KT * D], mybir.dt.bfloat16)
        kc_t = wpool.tile([P, KT * NK], mybir.dt.bfloat16)
        v_t = wpool.tile([P, D], mybir.dt.bfloat16)
        ident = wpool.tile([P, P], mybir.dt.bfloat16)
        # Split the big weight loads per k-tile so the first matmuls can
        # start as soon as their slice lands (sub-tile deps).
        for kt in range(KT):
            nc.sync.dma_start(wq_t[:, kt * D : (kt + 1) * D], wq_d[:, kt * D : (kt + 1) * D])
        nc.sync.dma_start(kc_t[:], kc_d[:])
        nc.sync.dma_start(v_t[:], v_d[:])
        for kt in range(KT):
            nc.sync.dma_start(wo_t[:, kt * D : (kt + 1) * D], wo_d[:, kt * D : (kt + 1) * D])
        make_identity(nc, ident[:])

        # Persistent transposed-probs buffer; pad rows zeroed once so the
        # K=128 PV matmul never multiplies v's zero pad rows by NaN garbage.
        expT = wpool.tile([P, H, CHUNK], mybir.dt.bfloat16)
        nc.gpsimd.memset(expT[:], 0.0)

        for c in [c for _ in range(reps) for c in range(N_CHUNKS)]:
            ht_t = hpool.tile([P, KT, CHUNK], mybir.dt.bfloat16)
            nc.sync.dma_start(ht_t[:], ht_view[:, :, c * CHUNK : (c + 1) * CHUNK])

            # ---- q projection: qT[:, mt, :] = Wq[:, mt-block]^T @ hiddenT ----
            qT_t = qpool.tile([P, KT, CHUNK], mybir.dt.bfloat16)
            for mt in range(KT):
                qp = accps.tile([P, CHUNK], mybir.dt.float32, tag="acc")
                for kt in range(KT):
                    nc.tensor.matmul(
                        qp[:],
                        lhsT=wq_t[:, kt * D + mt * P : kt * D + (mt + 1) * P],
                        rhs=ht_t[:, kt, :],
                        start=(kt == 0),
                        stop=(kt == KT - 1),
                    )
                nc.scalar.copy(qT_t[:, mt, :], qp[:])

            # ---- attention per head ----
            # 8 denominators per head: 4 q-tiles x (text, audio)
            dsum = dpool.tile([P, H * 8], mybir.dt.float32)
            rds = dpool.tile([P, H * 8], mybir.dt.float32, tag="rds")
            at_t = atpool.tile([P, KT, CHUNK], mybir.dt.bfloat16, tag="at")
            for g in range(H // 2):  # head pairs (2g, 2g+1)
                tpp = tps.tile([NK, 2 * CHUNK], mybir.dt.bfloat16)
                pvp = vps.tile([P, CHUNK], mybir.dt.float32)
                for hh in range(2):
                    h = 2 * g + hh
                    off = hh * HD
                    d0 = h * 8
                    sp = sps.tile([P, QT_PER_CHUNK * NK], mybir.dt.float32)
                    pu = ppool.tile([P, QT_PER_CHUNK * NK], mybir.dt.float32)
                    pn = pnpool.tile([P, QT_PER_CHUNK * NK], mybir.dt.bfloat16)
                    for qt in range(QT_PER_CHUNK):
                        nc.tensor.matmul(
                            sp[:, qt * NK : (qt + 1) * NK],
                            lhsT=qT_t[off : off + HD, g, qt * P : (qt + 1) * P],
                            rhs=kc_t[off : off + HD, g * NK : (g + 1) * NK],
                            start=True,
                            stop=True,
                        )
                    # one fused exp per head; denominators via segmented reduces
                    nc.scalar.activation(
                        pu[:],
                        sp[:],
                        mybir.ActivationFunctionType.Exp,
                        scale=float(SCALE),
                    )
                    pu3 = pu[:].rearrange("p (q k) -> p q k", q=QT_PER_CHUNK)
                    nc.vector.reduce_sum(
                        dsum[:, d0 : d0 + 4], pu3[:, :, 0:S_TXT],
                        axis=mybir.AxisListType.X,
                    )
                    nc.vector.reduce_sum(
                        dsum[:, d0 + 4 : d0 + 8], pu3[:, :, S_TXT:NK],
                        axis=mybir.AxisListType.X,
                    )
                    nc.vector.reciprocal(rds[:, d0 : d0 + 8], dsum[:, d0 : d0 + 8])
                    # batched normalize: one op per softmax, broadcasting the
                    # per-(partition, qtile) reciprocal along keys (step-0 AP)
                    pn3 = pn[:].rearrange("p (q k) -> p q k", q=QT_PER_CHUNK)
                    nc.vector.tensor_tensor(
                        pn3[:, :, 0:S_TXT],
                        pu3[:, :, 0:S_TXT],
                        rds[:, d0 : d0 + 4, None].to_broadcast([P, QT_PER_CHUNK, S_TXT]),
                        mybir.AluOpType.mult,
                    )
                    nc.vector.tensor_tensor(
                        pn3[:, :, S_TXT:NK],
                        pu3[:, :, S_TXT:NK],
                        rds[:, d0 + 4 : d0 + 8, None].to_broadcast([P, QT_PER_CHUNK, S_AUD]),
                        mybir.AluOpType.mult,
                    )
                    for qt in range(QT_PER_CHUNK):
                        nc.tensor.transpose(
                            tpp[:, hh * CHUNK + qt * P : hh * CHUNK + (qt + 1) * P],
                            pn[:, qt * NK : (qt + 1) * NK],
                            ident[:],
                        )
                # one copy moves both heads' transposed probs to SBUF
                nc.scalar.copy(expT[0:NK, 2 * g : 2 * g + 2, :], tpp[:])
                for hh in range(2):
                    h = 2 * g + hh
                    off = hh * HD
                    nc.tensor.matmul(
                        pvp[off : off + HD, :],
                        lhsT=v_t[:, h * HD : (h + 1) * HD],
                        rhs=expT[:, h, :],
                        start=True,
                        stop=True,
                    )
                # attn_T tile g holds heads 2g (rows 0-63) and 2g+1 (64-127)
                nc.scalar.copy(at_t[:, g, :], pvp[:])

            # ---- output projection: out[q, :] = attn_T.T @ Wo ----
            for qt in range(QT_PER_CHUNK):
                for nb in range(2):
                    op = accps.tile([P, CHUNK], mybir.dt.float32, tag="acc")
                    for kt in range(KT):
                        nc.tensor.matmul(
                            op[:],
                            lhsT=at_t[:, kt, qt * P : (qt + 1) * P],
                            rhs=wo_t[:, kt * D + nb * CHUNK : kt * D + (nb + 1) * CHUNK],
                            start=(kt == 0),
                            stop=(kt == KT - 1),
                        )
                    ob = opool.tile([P, CHUNK], mybir.dt.float32)
                    nc.vector.tensor_copy(ob[:], op[:])
                    nc.sync.dma_start(
                        out_d[
                            c * CHUNK + qt * P : c * CHUNK + (qt + 1) * P,
                            nb * CHUNK : (nb + 1) * CHUNK,
                        ],
                        ob[:],
                    )

    nc.compile()
    return nc


def _host_prep(hidden_states, encoder_hidden_states, audio_hidden_states,
               Wq, Wk, Wv, Wk_audio, Wv_audio, Wo):
    """Build the per-core input maps (all layouts pre-arranged on host)."""
    wq_sb = np.ascontiguousarray(
        Wq.reshape(KT, P, D).transpose(1, 0, 2).reshape(P, KT * D)
    ).astype(BF16)
    wo_sb = np.ascontiguousarray(
        Wo.reshape(KT, P, D).transpose(1, 0, 2).reshape(P, KT * D)
    ).astype(BF16)

    in_maps = []
    for b in range(B):
        # kv projections for this batch: tiny, done on host
        k_full = np.concatenate(
            [encoder_hidden_states[b] @ Wk, audio_hidden_states[b] @ Wk_audio], axis=0
        )  # [93, 1024]
        v_full = np.concatenate(
            [encoder_hidden_states[b] @ Wv, audio_hidden_states[b] @ Wv_audio], axis=0
        )  # [93, 1024]
        kc_sb = np.ascontiguousarray(
            k_full.T.reshape(KT, P, NK).transpose(1, 0, 2).reshape(P, KT * NK)
        ).astype(BF16)
        v_sb = np.zeros((P, D), dtype=BF16)
        v_sb[:NK] = v_full.astype(BF16)

        for half in range(2):
            rows = hidden_states[b, half * SEQ_PER_CORE : (half + 1) * SEQ_PER_CORE]
            ht_sb = np.ascontiguousarray(
                rows.T.reshape(KT, P, SEQ_PER_CORE)
                .transpose(1, 0, 2)
                .reshape(P, KT * SEQ_PER_CORE)
            ).astype(BF16)
            in_maps.append(
                {"ht": ht_sb, "wq": wq_sb, "wo": wo_sb, "kc": kc_sb, "v": v_sb}
            )
    return in_maps


_NC_CACHE = {}


def get_nc():
    if "nc" not in _NC_CACHE:
        _NC_CACHE["nc"] = build_bass()
    return _NC_CACHE["nc"]


def kernel(hidden_states, encoder_hidden_states, audio_hidden_states,
           Wq, Wk, Wv, Wk_audio, Wv_audio, Wo, bo):
    from concourse import bass_utils

    hidden_states = np.asarray(hidden_states, dtype=np.float32)
    encoder_hidden_states = np.asarray(encoder_hidden_states, dtype=np.float32)
    audio_hidden_states = np.asarray(audio_hidden_states, dtype=np.float32)
    Wq = np.asarray(Wq, dtype=np.float32)
    Wk = np.asarray(Wk, dtype=np.float32)
    Wv = np.asarray(Wv, dtype=np.float32)
    Wk_audio = np.asarray(Wk_audio, dtype=np.float32)
    Wv_audio = np.asarray(Wv_audio, dtype=np.float32)
    Wo = np.asarray(Wo, dtype=np.float32)
    bo = np.asarray(bo, dtype=np.float32)

    nc = get_nc()
    in_maps = _host_prep(hidden_states, encoder_hidden_states, audio_hidden_states,
                         Wq, Wk, Wv, Wk_audio, Wv_audio, Wo)
    res = bass_utils.run_bass_kernel_spmd(nc, in_maps, list(range(N_CORES)))

    out = np.empty((B, S, D), dtype=np.float32)
    core = 0
    for b in range(B):
        for half in range(2):
            out[b, half * SEQ_PER_CORE : (half + 1) * SEQ_PER_CORE] = res.results[core]["out"]
            core += 1
    out += bo[None, None, :]
    return out


# revision 28
# speedup vs baseline: 259.8453x; 1.2432x over previous
"""Trainium2 Bass kernel for AudioAdapterAttnProcessor.

Reference computation (B=4, S=4096, D=1024, H=16, HD=64, C=768,
S_TXT=77, S_AUD=16):
    q = (hidden @ Wq)                                  [B, S, H, HD]
    base  = softmax(q k_t^T / 8) v_t   (text cross-attn, k/v from encoder)
    audio = softmax(q k_a^T / 8) v_a   (audio cross-attn)
    out = concat_heads(base + audio) @ Wo + bo

Sharding: (batch x seq-half) -> 8 cores; each core handles one batch's
2048 queries for all 16 heads.  No collectives: every query row of the
output depends only on its own hidden row (cross-attention to 93 fixed
keys per batch), so the gather is a pure concatenation.

Per-core device program (everything transposed so PE contracts naturally):
    qT   = Wq^T @ hiddenT                (hiddenT fed pre-transposed, bf16)
    s    = q_h^T.T @ kcatT_h             natural scores [128q, 93keys]
    p    = exp(s/8) (fused scale); denominators via ACT accum_out
    pn   = p * recip(den)                per-partition tensor_scalar
    pT   = PE-transpose(pn)              [93, 128] bf16
    pv   = v_h(pad128)^T.T @ pT          -> attn_outT [64, 512] per head
    out  = attn_T.T @ Wo                 natural [128q, 1024] fp32 -> DRAM
bo is added on the host during the gather (it is per-output-feature,
which is the free dim of the natural-layout output).
"""

import sys

sys.path.insert(0, "/opt/trn_rl_repo")

from contextlib import ExitStack

import numpy as np
import ml_dtypes

import concourse.bass as bass
import concourse.mybir as mybir
import concourse.tile as tile
from concourse import bacc
from concourse.masks import make_identity

BF16 = ml_dtypes.bfloat16

B, S, D = 4, 4096, 1024
S_TXT, S_AUD = 77, 16
C = 768
H = 16
HD = 64
NK = S_TXT + S_AUD  # 93 keys after concat
P = 128
SCALE = 1.0 / np.sqrt(HD)  # 0.125

N_CORES = 8
SEQ_PER_CORE = S // 2  # 2048
CHUNK = 512
N_CHUNKS = SEQ_PER_CORE // CHUNK  # 4
KT = D // P  # 8 contraction tiles
QT_PER_CHUNK = CHUNK // P  # 4


DEFAULT_CFG = dict(hidden=2, qT=2, probs_u=3, probs_n=3, dsum=2, attn_T=2,
                   out_sb=3, accps=4, sps=2, tps=1, vps=1,
                   prio_qT=0, prio_expT=0, prio_at=0, qT_dve=False,
                   at_dve=False, expT_dve=False)


def build_bass(cfg=None, reps=1):
    """Build the SPMD single-core Bass program (same program on all 8 cores).

    reps > 1 repeats the whole computation back-to-back inside the NEFF
    (same inputs -> same outputs); used only for slope-based timing.
    """
    cfg = {**DEFAULT_CFG, **(cfg or {})}
    nc = bacc.Bacc("TRN2", target_bir_lowering=False, debug=False, num_devices=N_CORES)

    ht_d = nc.dram_tensor("ht", [P, KT * SEQ_PER_CORE], mybir.dt.bfloat16, kind="ExternalInput")
    wq_d = nc.dram_tensor("wq", [P, KT * D], mybir.dt.bfloat16, kind="ExternalInput")
    wo_d = nc.dram_tensor("wo", [P, KT * D], mybir.dt.bfloat16, kind="ExternalInput")
    kc_d = nc.dram_tensor("kc", [P, KT * NK], mybir.dt.bfloat16, kind="ExternalInput")
    v_d = nc.dram_tensor("v", [P, D], mybir.dt.bfloat16, kind="ExternalInput")
    out_d = nc.dram_tensor("out", [SEQ_PER_CORE, D], mybir.dt.float32, kind="ExternalOutput")

    ht_view = ht_d[:].rearrange("p (k q) -> p k q", k=KT)  # [128, 8, 2048]

    import contextlib

    with tile.TileContext(nc) as tc, ExitStack() as ctx:
        def gate_copy(dst, src, prio, on_dve):
            """PSUM->SBUF copy that gates PE work; optionally boosted/moved."""
            cm = (tc.high_priority(None if prio < 0 else prio)
                  if prio else contextlib.nullcontext())
            with cm:
                if on_dve:
                    nc.vector.tensor_copy(dst, src)
                else:
                    nc.scalar.copy(dst, src)

        wpool = ctx.enter_context(tc.tile_pool(name="weights", bufs=1))
        hpool = ctx.enter_context(tc.tile_pool(name="hidden", bufs=cfg["hidden"]))
        qpool = ctx.enter_context(tc.tile_pool(name="qT", bufs=cfg["qT"]))
        ppool = ctx.enter_context(tc.tile_pool(name="probs_u", bufs=cfg["probs_u"]))
        pnpool = ctx.enter_context(tc.tile_pool(name="probs_n", bufs=cfg["probs_n"]))
        dpool = ctx.enter_context(tc.tile_pool(name="dsum", bufs=cfg["dsum"]))
        atpool = ctx.enter_context(tc.tile_pool(name="attn_T", bufs=cfg["attn_T"]))
        opool = ctx.enter_context(tc.tile_pool(name="out_sb", bufs=cfg["out_sb"]))

        accps = ctx.enter_context(tc.tile_pool(name="accps", bufs=cfg["accps"], space="PSUM"))
        sps = ctx.enter_context(tc.tile_pool(name="sps", bufs=cfg["sps"], space="PSUM"))
        tps = ctx.enter_context(tc.tile_pool(name="tps", bufs=cfg["tps"], space="PSUM"))
        vps = ctx.enter_context(tc.tile_pool(name="vps", bufs=cfg["vps"], space="PSUM"))

        # One-time loads
        wq_t = wpool.tile([P, KT * D], mybir.dt.bfloat16)
        wo_t = wpool.tile([P, KT * D], mybir.dt.bfloat16)
        kc_t = wpool.tile([P, KT * NK], mybir.dt.bfloat16)
        v_t = wpool.tile([P, D], mybir.dt.bfloat16)
        ident = wpool.tile([P, P], mybir.dt.bfloat16)
        # DMA issue order matters: the HWDGE queue drains in order, so put
        # everything the first q-projection needs ahead of the 2MB wo load.
        # Per-k-tile splits let matmuls start on sub-tile deps.
        ht_t0 = hpool.tile([P, KT, CHUNK], mybir.dt.bfloat16, tag="ht")
        for kt in range(KT):
            nc.sync.dma_start(wq_t[:, kt * D : (kt + 1) * D], wq_d[:, kt * D : (kt + 1) * D])
            nc.sync.dma_start(ht_t0[:, kt, :], ht_view[:, kt, 0:CHUNK])
        nc.sync.dma_start(kc_t[:], kc_d[:])
        nc.sync.dma_start(v_t[:], v_d[:])
        for kt in range(KT):
            nc.sync.dma_start(wo_t[:, kt * D : (kt + 1) * D], wo_d[:, kt * D : (kt + 1) * D])
        make_identity(nc, ident[:])

        # Persistent transposed-probs buffer; pad rows zeroed once so the
        # K=128 PV matmul never multiplies v's zero pad rows by NaN garbage.
        expT = wpool.tile([P, H, CHUNK], mybir.dt.bfloat16)
        nc.gpsimd.memset(expT[:], 0.0)

        def emit_ht(c, tile_=None):
            ht_t = tile_ or hpool.tile([P, KT, CHUNK], mybir.dt.bfloat16, tag="ht")
            if tile_ is None:
                nc.sync.dma_start(ht_t[:], ht_view[:, :, c * CHUNK : (c + 1) * CHUNK])
            return ht_t

        def emit_qproj_mt(ht_t, qT_t, mt):
            qp = accps.tile([P, CHUNK], mybir.dt.float32, tag="acc")
            for kt in range(KT):
                nc.tensor.matmul(
                    qp[:],
                    lhsT=wq_t[:, kt * D + mt * P : kt * D + (mt + 1) * P],
                    rhs=ht_t[:, kt, :],
                    start=(kt == 0),
                    stop=(kt == KT - 1),
                )
            gate_copy(qT_t[:, mt, :], qp[:], cfg["prio_qT"], cfg["qT_dve"])

        def emit_attention(c, qT_t, interleave=None):
            # 8 denominators per head: 4 q-tiles x (text, audio)
            dsum = dpool.tile([P, H * 8], mybir.dt.float32)
            rds = dpool.tile([P, H * 8], mybir.dt.float32, tag="rds")
            at_t = atpool.tile([P, KT, CHUNK], mybir.dt.bfloat16, tag="at")
            for g in range(H // 2):  # head pairs (2g, 2g+1)
                tpp = tps.tile([NK, 2 * CHUNK], mybir.dt.bfloat16)
                pvp = vps.tile([P, CHUNK], mybir.dt.float32)
                for hh in range(2):
                    h = 2 * g + hh
                    off = hh * HD
                    d0 = h * 8
                    sp = sps.tile([P, QT_PER_CHUNK * NK], mybir.dt.float32)
                    pu = ppool.tile([P, QT_PER_CHUNK * NK], mybir.dt.float32)
                    pn = pnpool.tile([P, QT_PER_CHUNK * NK], mybir.dt.bfloat16)
                    for qt in range(QT_PER_CHUNK):
                        nc.tensor.matmul(
                            sp[:, qt * NK : (qt + 1) * NK],
                            lhsT=qT_t[off : off + HD, g, qt * P : (qt + 1) * P],
                            rhs=kc_t[off : off + HD, g * NK : (g + 1) * NK],
                            start=True,
                            stop=True,
                        )
                    # one fused exp per head; denominators via segmented reduces
                    nc.scalar.activation(
                        pu[:],
                        sp[:],
                        mybir.ActivationFunctionType.Exp,
                        scale=float(SCALE),
                    )
                    pu3 = pu[:].rearrange("p (q k) -> p q k", q=QT_PER_CHUNK)
                    nc.vector.reduce_sum(
                        dsum[:, d0 : d0 + 4], pu3[:, :, 0:S_TXT],
                        axis=mybir.AxisListType.X,
                    )
                    nc.vector.reduce_sum(
                        dsum[:, d0 + 4 : d0 + 8], pu3[:, :, S_TXT:NK],
                        axis=mybir.AxisListType.X,
                    )
                    nc.vector.reciprocal(rds[:, d0 : d0 + 8], dsum[:, d0 : d0 + 8])
                    # batched normalize: one op per softmax, broadcasting the
                    # per-(partition, qtile) reciprocal along keys (step-0 AP)
                    pn3 = pn[:].rearrange("p (q k) -> p q k", q=QT_PER_CHUNK)
                    nc.vector.tensor_tensor(
                        pn3[:, :, 0:S_TXT],
                        pu3[:, :, 0:S_TXT],
                        rds[:, d0 : d0 + 4, None].to_broadcast([P, QT_PER_CHUNK, S_TXT]),
                        mybir.AluOpType.mult,
                    )
                    nc.vector.tensor_tensor(
                        pn3[:, :, S_TXT:NK],
                        pu3[:, :, S_TXT:NK],
                        rds[:, d0 + 4 : d0 + 8, None].to_broadcast([P, QT_PER_CHUNK, S_AUD]),
                        mybir.AluOpType.mult,
                    )
                    for qt in range(QT_PER_CHUNK):
                        nc.tensor.transpose(
                            tpp[:, hh * CHUNK + qt * P : hh * CHUNK + (qt + 1) * P],
                            pn[:, qt * NK : (qt + 1) * NK],
                            ident[:],
                        )
                # one copy moves both heads' transposed probs to SBUF
                gate_copy(expT[0:NK, 2 * g : 2 * g + 2, :], tpp[:],
                          cfg["prio_expT"], cfg["expT_dve"])
                for hh in range(2):
                    h = 2 * g + hh
                    off = hh * HD
                    nc.tensor.matmul(
                        pvp[off : off + HD, :],
                        lhsT=v_t[:, h * HD : (h + 1) * HD],
                        rhs=expT[:, h, :],
                        start=True,
                        stop=True,
                    )
                # attn_T tile g holds heads 2g (rows 0-63) and 2g+1 (64-127)
                gate_copy(at_t[:, g, :], pvp[:], cfg["prio_at"], cfg["at_dve"])
                if interleave is not None:
                    interleave(g)
            return at_t

        def emit_outproj(c, at_t):
            for qt in range(QT_PER_CHUNK):
                for nb in range(2):
                    op = accps.tile([P, CHUNK], mybir.dt.float32, tag="acc")
                    for kt in range(KT):
                        nc.tensor.matmul(
                            op[:],
                            lhsT=at_t[:, kt, qt * P : (qt + 1) * P],
                            rhs=wo_t[:, kt * D + nb * CHUNK : kt * D + (nb + 1) * CHUNK],
                            start=(kt == 0),
                            stop=(kt == KT - 1),
                        )
                    ob = opool.tile([P, CHUNK], mybir.dt.float32)
                    nc.vector.tensor_copy(ob[:], op[:])
                    nc.sync.dma_start(
                        out_d[
                            c * CHUNK + qt * P : c * CHUNK + (qt + 1) * P,
                            nb * CHUNK : (nb + 1) * CHUNK,
                        ],
                        ob[:],
                    )

        chunks = [c for _ in range(reps) for c in range(N_CHUNKS)]
        mode = cfg.get("interleave", 2)
        if mode == 0:
            for i, c in enumerate(chunks):
                ht_t = emit_ht(c, ht_t0 if i == 0 else None)
                qT_t = qpool.tile([P, KT, CHUNK], mybir.dt.bfloat16)
                for mt in range(KT):
                    emit_qproj_mt(ht_t, qT_t, mt)
                at_t = emit_attention(c, qT_t)
                emit_outproj(c, at_t)
        else:
            # software-pipelined emission: qproj(c+1) interleaves with the
            # attention phase of chunk c (one mt-group per head pair).
            ht_t = emit_ht(chunks[0], ht_t0)
            qT_t = qpool.tile([P, KT, CHUNK], mybir.dt.bfloat16)
            for mt in range(KT):
                emit_qproj_mt(ht_t, qT_t, mt)
            for i, c in enumerate(chunks):
                nxt = chunks[i + 1] if i + 1 < len(chunks) else None
                if nxt is not None:
                    ht_next = emit_ht(nxt)
                    qT_next = qpool.tile([P, KT, CHUNK], mybir.dt.bfloat16)
                    def ilv(g, ht_next=ht_next, qT_next=qT_next):
                        emit_qproj_mt(ht_next, qT_next, g)
                else:
                    ilv = None
                at_t = emit_attention(c, qT_t, interleave=ilv)
                emit_outproj(c, at_t)
                if nxt is not None:
                    qT_t = qT_next

    nc.compile()
    return nc


def _host_prep(hidden_states, encoder_hidden_states, audio_hidden_states,
               Wq, Wk, Wv, Wk_audio, Wv_audio, Wo):
    """Build the per-core input maps (all layouts pre-arranged on host)."""
    wq_sb = np.ascontiguousarray(
        Wq.reshape(KT, P, D).transpose(1, 0, 2).reshape(P, KT * D)
    ).astype(BF16)
    wo_sb = np.ascontiguousarray(
        Wo.reshape(KT, P, D).transpose(1, 0, 2).reshape(P, KT * D)
    ).astype(BF16)

    in_maps = []
    for b in range(B):
        # kv projections for this batch: tiny, done on host
        k_full = np.concatenate(
            [encoder_hidden_states[b] @ Wk, audio_hidden_states[b] @ Wk_audio], axis=0
        )  # [93, 1024]
        v_full = np.concatenate(
            [encoder_hidden_states[b] @ Wv, audio_hidden_states[b] @ Wv_audio], axis=0
        )  # [93, 1024]
        kc_sb = np.ascontiguousarray(
            k_full.T.reshape(KT, P, NK).transpose(1, 0, 2).reshape(P, KT * NK)
        ).astype(BF16)
        v_sb = np.zeros((P, D), dtype=BF16)
        v_sb[:NK] = v_full.astype(BF16)

        for half in range(2):
            rows = hidden_states[b, half * SEQ_PER_CORE : (half + 1) * SEQ_PER_CORE]
            ht_sb = np.ascontiguousarray(
                rows.T.reshape(KT, P, SEQ_PER_CORE)
                .transpose(1, 0, 2)
                .reshape(P, KT * SEQ_PER_CORE)
            ).astype(BF16)
            in_maps.append(
                {"ht": ht_sb, "wq": wq_sb, "wo": wo_sb, "kc": kc_sb, "v": v_sb}
            )
    return in_maps


_NC_CACHE = {}


def get_nc():
    if "nc" not in _NC_CACHE:
        _NC_CACHE["nc"] = build_bass()
    return _NC_CACHE["nc"]


def kernel(hidden_states, encoder_hidden_states, audio_hidden_states,
           Wq, Wk, Wv, Wk_audio, Wv_audio, Wo, bo):
    from concourse import bass_utils

    hidden_states = np.asarray(hidden_states, dtype=np.float32)
    encoder_hidden_states = np.asarray(encoder_hidden_states, dtype=np.float32)
    audio_hidden_states = np.asarray(audio_hidden_states, dtype=np.float32)
    Wq = np.asarray(Wq, dtype=np.float32)
    Wk = np.asarray(Wk, dtype=np.float32)
    Wv = np.asarray(Wv, dtype=np.float32)
    Wk_audio = np.asarray(Wk_audio, dtype=np.float32)
    Wv_audio = np.asarray(Wv_audio, dtype=np.float32)
    Wo = np.asarray(Wo, dtype=np.float32)
    bo = np.asarray(bo, dtype=np.float32)

    nc = get_nc()
    in_maps = _host_prep(hidden_states, encoder_hidden_states, audio_hidden_states,
                         Wq, Wk, Wv, Wk_audio, Wv_audio, Wo)
    res = bass_utils.run_bass_kernel_spmd(nc, in_maps, list(range(N_CORES)))

    out = np.empty((B, S, D), dtype=np.float32)
    core = 0
    for b in range(B):
        for half in range(2):
            out[b, half * SEQ_PER_CORE : (half + 1) * SEQ_PER_CORE] = res.results[core]["out"]
            core += 1
    out += bo[None, None, :]
    return out
